# revision 5
# baseline (speedup 1.0000x reference)
"""Trainium2 Bass kernel for nn_CAGKE_1 (Gaussian-kernel embedding).

Math: reference computes, for mask m_i = 1[X_i > 0.5],
    out[j] = sum_e softmax(w)_e * sum_i m_i * (c/sigma_e) exp(-(j-i-1)^2/(2 sigma_e^2)) + noise_j
Both sums are linear, so the E=128 Gaussian channels collapse into one
combined kernel ghat(d) = sum_e softmax(w)_e * (c/sigma_e) exp(-d^2/(2 sigma_e^2))
BEFORE the convolution; 255 taps (|d| <= 127ish) are exact at f32 precision.

v2: the conv is a banded-Toeplitz matmul, and the Toeplitz factor now lives
in the MASK operand, whose expansion is pure host-side layout of the input:
the host sends 9 blocks M_beta[u, r] = Xwin[128*beta + r - u + 126] (raw X
values; device binarizes with >0.5). ghat is produced directly as a COLUMN
G[u, gamma] = ghat(u + 128*gamma - 127) by two matmuls with the per-sigma exp
table as the stationary operand, so there is no DRAM round-trip and no PE
transpose anywhere:

  out^T[r, b] = sum_{gamma=0,1} sum_u M_{b+2-gamma}[u, r] * G[u, gamma] + noise^T

  - softmax is computed max-free on a w COLUMN: Z broadcast to all
    partitions via a matmul with an all-(1/c) stationary, so the whole
    weight chain is 1 ACT + 1 matmul + 3 tiny vector ops.
  - PSUM is zeroed by one early matmul with a zeros moving operand; the 9
    conv matmuls then accumulate into column slices.
  - inputs are split into 4 DMAs (1 small f32 parcel + 3 Toeplitz chunks)
    issued from Scalar/Sync/GpSimd right after the preamble so flight time
    overlaps the activation-table load.
"""

import sys

import numpy as np

if "/opt/trn_rl_repo" not in sys.path:
    sys.path.insert(0, "/opt/trn_rl_repo")

T = 8192
E = 128
N_CORES = 8
TJ = T // N_CORES          # 1024 outputs per core
NB = TJ // 128             # 8 output blocks of 128
NBETA = 9                  # mask-Toeplitz blocks beta = 1..9
MW = NBETA * 128           # 1152 mask-Toeplitz columns
LK = 256                   # ghat taps, d = t - 127 for t in [0, 256)
INV_SQRT_2PI = 0.39894228

_compiled = None


def _build():
    import concourse.bacc as bacc
    import concourse.bass as bass
    import concourse.mybir as mybir
    import concourse.tile as tile

    f32 = mybir.dt.float32
    nc = bacc.Bacc(num_devices=N_CORES, debug=False)

    parcel_d = nc.dram_tensor("parcel", [128, NB + 2], f32, kind="ExternalInput")
    mtoep_d = nc.dram_tensor("mtoep", [128, MW], f32, kind="ExternalInput")
    out_d = nc.dram_tensor("out", [128, NB], f32, kind="ExternalOutput")

    with tile.TileContext(nc) as tc:
        with (
            tc.tile_pool(name="pool", bufs=1) as pool,
            tc.tile_pool(name="psum", bufs=1, space="PSUM") as psum,
        ):
            # ---- input loads, split across issue engines ----
            parcel = pool.tile([128, NB + 2], f32, tag="parcel")
            nc.scalar.dma_start(parcel[:], parcel_d[:])
            mraw = pool.tile([128, MW], f32, tag="mraw")
            nc.gpsimd.dma_start(mraw[:, 768:1152], mtoep_d[:, 768:1152])
            nc.sync.dma_start(mraw[:, 0:384], mtoep_d[:, 0:384])
            nc.sync.dma_start(mraw[:, 384:768], mtoep_d[:, 384:768])
            nzT = parcel[:, 0:NB]
            sgT = parcel[:, NB : NB + 1]
            wT = parcel[:, NB + 1 : NB + 2]

            # ---- input-independent prep ----
            zeros8 = pool.tile([128, NB], f32, tag="zeros8")
            nc.vector.memset(zeros8[:], 0.0)
            onesInv = pool.tile([128, 128], f32, tag="onesInv")
            nc.vector.memset(onesInv[:], 1.0 / INV_SQRT_2PI)
            dlt = pool.tile([128, LK], f32, tag="dlt")
            nc.gpsimd.iota(
                dlt[:], pattern=[[1, LK]], base=-127, channel_multiplier=0,
                allow_small_or_imprecise_dtypes=True,
            )
            d2n = pool.tile([128, LK], f32, tag="d2n")
            nc.gpsimd.tensor_mul(d2n[:], dlt[:], dlt[:])
            nc.gpsimd.tensor_scalar_mul(d2n[:], d2n[:], -0.5)

            op = psum.tile([128, NB], f32, tag="op")

            # ---- sigma column chain (starts as soon as parcel lands) ----
            s2 = pool.tile([128, 1], f32, tag="s2")
            nc.vector.tensor_mul(s2[:], sgT, sgT)
            invs = pool.tile([128, 1], f32, tag="invs")
            nc.vector.reciprocal(invs[:], s2[:])          # 1/sigma^2
            rs = pool.tile([128, 1], f32, tag="rs")
            nc.vector.reciprocal(rs[:], sgT)              # 1/sigma

            # ---- softmax, max-free, all in column space ----
            exp_col = pool.tile([128, 1], f32, tag="exp_col")
            nc.scalar.activation(exp_col[:], wT, mybir.ActivationFunctionType.Exp)
            zp = psum.tile([128, 1], f32, tag="zp")
            nc.tensor.matmul(zp[:], onesInv[:], exp_col[:], start=True, stop=True)
            czr = pool.tile([128, 1], f32, tag="czr")
            nc.vector.reciprocal(czr[:], zp[:])           # c / Z on every partition
            t1 = pool.tile([128, 1], f32, tag="t1")
            nc.vector.tensor_mul(t1[:], exp_col[:], rs[:])
            a = pool.tile([128, 1], f32, tag="a")
            nc.vector.tensor_mul(a[:], t1[:], czr[:])     # softmax(w)*c/sigma

            # ---- per-sigma exp table, e on partitions ----
            expt = pool.tile([128, LK], f32, tag="expt")
            nc.scalar.activation(
                expt[:], d2n[:], mybir.ActivationFunctionType.Exp, scale=invs[:]
            )

            # ---- ghat as a column: G[u, g] = ghat(u + 128 g - 127) ----
            gp = psum.tile([128, 2], f32, tag="gp")
            nc.tensor.matmul(gp[:, 0:1], expt[:, 0:128], a[:], start=True, stop=True)
            nc.tensor.matmul(gp[:, 1:2], expt[:, 128:256], a[:], start=True, stop=True)
            gs = pool.tile([128, 2], f32, tag="gs")
            nc.vector.tensor_scalar(gs[:], gp[:], 1.0, None, mybir.AluOpType.mult)

            # ---- binarize mask-Toeplitz blocks (chunked to chase the DMAs) ----
            mb = pool.tile([128, MW], f32, tag="mb")
            nc.gpsimd.tensor_scalar(
                mb[:, 768:1152], mraw[:, 768:1152], 0.5, None, mybir.AluOpType.is_gt
            )
            nc.vector.tensor_scalar(
                mb[:, 0:384], mraw[:, 0:384], 0.5, None, mybir.AluOpType.is_gt
            )
            nc.gpsimd.tensor_scalar(
                mb[:, 384:768], mraw[:, 384:768], 0.5, None, mybir.AluOpType.is_gt
            )

            # ---- conv: 9 accumulating banded-Toeplitz matmuls ----
            # zero the output PSUM bank via one matmul (PE-ordered right
            # before the group so no other matmul interleaves it); the 9
            # conv matmuls then pure-accumulate in any column order
            nc.tensor.matmul(op[:], onesInv[:], zeros8[:], start=True, stop=False)
            # op[r, b] += sum_u M_beta[u, r] * G[u, g] with b = beta - 2 + g
            for beta in range(1, NBETA + 1):
                mslice = mb[:, 128 * (beta - 1) : 128 * beta]
                if beta == 1:
                    o, g = op[:, 0:1], gs[:, 1:2]
                elif beta == NBETA:
                    o, g = op[:, NB - 1 : NB], gs[:, 0:1]
                else:
                    o, g = op[:, beta - 2 : beta], gs[:, 0:2]
                # middle matmuls skip the sim's bank-granular group check
                # (their slices don't cover the whole started region); the
                # last one keeps it so stop=True closes the group.
                nc.tensor.matmul(
                    o, mslice, g, start=False, stop=(beta == NBETA),
                    skip_group_check=(beta != NBETA),
                )

            # ---- add noise (fused with PSUM read), store ----
            outS = pool.tile([128, NB], f32, tag="outS")
            nc.vector.tensor_add(outS[:], op[:], nzT)
            nc.sync.dma_start(out_d[:], outS[:])

    nc.compile()
    return nc


def kernel(X, sigma, weight, noise):
    global _compiled
    from concourse.bass_utils import run_bass_kernel_spmd

    X = np.ascontiguousarray(np.asarray(X, dtype=np.float32)).reshape(1, T)
    sigma = np.ascontiguousarray(np.asarray(sigma, dtype=np.float32)).reshape(E)
    weight = np.ascontiguousarray(np.asarray(weight, dtype=np.float32)).reshape(1, E)
    noise = np.ascontiguousarray(np.asarray(noise, dtype=np.float32)).reshape(1, T)

    if _compiled is None:
        _compiled = _build()
    nc = _compiled

    # mask-Toeplitz expansion (layout only; binarization happens on device):
    # M_beta[u, r] = Xpad[256 + c*1024 + 128*beta + 126 - u + r - 256]
    Xpad = np.zeros(256 + T + 512, dtype=np.float32)
    Xpad[256 : 256 + T] = X[0]
    sw = np.lib.stride_tricks.sliding_window_view(Xpad, 128)
    u = np.arange(128)[:, None]
    betas = 128 * np.arange(1, NBETA + 1)[None, :]
    in_maps = []
    for c in range(N_CORES):
        idx = c * TJ + 126 + betas - u                      # [128, 9]
        mtoep = np.ascontiguousarray(sw[idx].reshape(128, MW))
        parcel = np.empty((128, NB + 2), dtype=np.float32)
        parcel[:, 0:NB] = noise[0, c * TJ : (c + 1) * TJ].reshape(NB, 128).T
        parcel[:, NB] = sigma
        parcel[:, NB + 1] = weight[0]
        in_maps.append({"parcel": parcel, "mtoep": mtoep})

    res = run_bass_kernel_spmd(nc, in_maps, core_ids=list(range(N_CORES)))
    out = np.empty((1, T), dtype=np.float32)
    for c in range(N_CORES):
        out[0, c * TJ : (c + 1) * TJ] = res.results[c]["out"].T.reshape(-1)
    return out


# revision 6
# speedup vs baseline: 1.3503x; 1.3503x over previous
"""Trainium2 Bass kernel for nn_CAGKE_1 (Gaussian-kernel embedding).

Math: reference computes, for mask m_i = 1[X_i > 0.5],
    out[j] = sum_e softmax(w)_e * sum_i m_i * (c/sigma_e) exp(-(j-i-1)^2/(2 sigma_e^2)) + noise_j
Both sums are linear, so the E=128 Gaussian channels collapse into one
combined kernel ghat(d) = sum_e softmax(w)_e * (c/sigma_e) exp(-d^2/(2 sigma_e^2))
BEFORE the convolution; 255 taps (|d| <= 127ish) are exact at f32 precision.

The conv is a banded-Toeplitz matmul whose Toeplitz factor lives in the MASK
operand: the host sends 9 blocks M_beta[u, r] = Xwin[128*beta + r - u + 126]
- 0.5 as bf16 (pure layout + affine shift; the sign of X-0.5 is exactly
preserved by bf16 rounding, so the device's >0 binarize reproduces X>0.5
bit-exactly). ghat is produced directly as a COLUMN G[u, g] = ghat(u + 128g
- 127) by two matmuls with the per-sigma exp table as the stationary
operand, so there is no DRAM round-trip and no PE transpose anywhere:

  out^T[r, b] = sum_{g=0,1} sum_u M_{b+2-g}[u, r] * G[u, g] + noise^T

  - softmax is computed max-free on a w COLUMN: Z is broadcast to all
    partitions by a matmul with an all-(1/c) stationary; 1/Z is folded into
    the PSUM->SBUF copy of G, so nothing softmax-related gates the exp table.
  - PSUM is zeroed by one early matmul with a zeros moving operand; the 9
    conv matmuls then pure-accumulate into column slices.
  - binarize is chunked into 6 small pieces split across DVE/GpSimd so no
    single piece can head-of-line-block the critical vector chain.
  - all DMAs ride hardware DGE (Scalar: parcel, Sync: mask chunks + store);
    the GpSimd software DGE is ~13 GB/s and stalls the whole core.
"""

import sys

import numpy as np

if "/opt/trn_rl_repo" not in sys.path:
    sys.path.insert(0, "/opt/trn_rl_repo")

T = 8192
E = 128
N_CORES = 8
TJ = T // N_CORES          # 1024 outputs per core
NB = TJ // 128             # 8 output blocks of 128
NBETA = 9                  # mask-Toeplitz blocks beta = 1..9
MW = NBETA * 128           # 1152 mask-Toeplitz columns
LK = 256                   # ghat taps, d = t - 127 for t in [0, 256)
INV_SQRT_2PI = 0.39894228

_compiled = None


def _build():
    import concourse.bacc as bacc
    import concourse.bass as bass
    import concourse.mybir as mybir
    import concourse.tile as tile

    f32 = mybir.dt.float32
    bf16 = mybir.dt.bfloat16
    nc = bacc.Bacc(num_devices=N_CORES, debug=False)

    parcel_d = nc.dram_tensor("parcel", [128, NB + 2], f32, kind="ExternalInput")
    mtoep_d = nc.dram_tensor("mtoep", [128, MW], bf16, kind="ExternalInput")
    out_d = nc.dram_tensor("out", [128, NB], f32, kind="ExternalOutput")

    with tile.TileContext(nc) as tc:
        with (
            tc.tile_pool(name="pool", bufs=1) as pool,
            tc.tile_pool(name="psum", bufs=1, space="PSUM") as psum,
        ):
            # ---- input loads: parcel on Scalar-HWDGE, mask on Sync-HWDGE ----
            parcel = pool.tile([128, NB + 2], f32, tag="parcel")
            nc.scalar.dma_start(parcel[:], parcel_d[:])
            mraw = pool.tile([128, MW], bf16, tag="mraw")
            nc.sync.dma_start(mraw[:, 0:576], mtoep_d[:, 0:576])
            nc.sync.dma_start(mraw[:, 576:MW], mtoep_d[:, 576:MW])
            nzT = parcel[:, 0:NB]
            sgT = parcel[:, NB : NB + 1]
            wT = parcel[:, NB + 1 : NB + 2]

            # ---- input-independent prep ----
            zeros8 = pool.tile([128, NB], f32, tag="zeros8")
            nc.vector.memset(zeros8[:], 0.0)
            onesInv = pool.tile([128, 128], f32, tag="onesInv")
            nc.vector.memset(onesInv[:], 1.0 / INV_SQRT_2PI)
            dlt = pool.tile([128, LK], f32, tag="dlt")
            nc.gpsimd.iota(
                dlt[:], pattern=[[1, LK]], base=-127, channel_multiplier=0,
                allow_small_or_imprecise_dtypes=True,
            )
            d2 = pool.tile([128, LK], f32, tag="d2")
            nc.gpsimd.tensor_mul(d2[:], dlt[:], dlt[:])

            # zero the output PSUM bank right away; conv matmuls accumulate.
            # (its bank's accumulation group stays open while zp/gp run in
            # their own banks, which hardware and sim both allow)
            op = psum.tile([128, NB], f32, tag="op")
            nc.tensor.matmul(op[:], onesInv[:], zeros8[:], start=True, stop=False)

            # ---- sigma column chain (starts as soon as parcel lands) ----
            s2 = pool.tile([128, 1], f32, tag="s2")
            nc.vector.tensor_mul(s2[:], sgT, sgT)
            nc.vector.tensor_scalar_mul(s2[:], s2[:], -2.0)
            invs = pool.tile([128, 1], f32, tag="invs")
            nc.vector.reciprocal(invs[:], s2[:])          # -1/(2 sigma^2)
            rs = pool.tile([128, 1], f32, tag="rs")
            nc.vector.reciprocal(rs[:], sgT)              # 1/sigma

            # ---- softmax pieces, max-free, all in column space ----
            exp_col = pool.tile([128, 1], f32, tag="exp_col")
            nc.scalar.activation(exp_col[:], wT, mybir.ActivationFunctionType.Exp)
            t1 = pool.tile([128, 1], f32, tag="t1")
            nc.vector.tensor_mul(t1[:], exp_col[:], rs[:])   # exp(w)/sigma
            zp = psum.tile([128, 1], f32, tag="zp")
            nc.tensor.matmul(zp[:], onesInv[:], exp_col[:], start=True, stop=True)
            czr = pool.tile([128, 1], f32, tag="czr")
            nc.vector.reciprocal(czr[:], zp[:])           # c/Z on every partition

            # ---- per-sigma exp table, e on partitions ----
            expt = pool.tile([128, LK], f32, tag="expt")
            nc.scalar.activation(
                expt[:], d2[:], mybir.ActivationFunctionType.Exp, scale=invs[:]
            )

            # ---- ghat as a column: G[u, g] = ghat(u + 128 g - 127) ----
            # (1/Z folded into the PSUM->SBUF copy, bf16 convert for the conv)
            gp = psum.tile([128, 2], f32, tag="gp")
            nc.tensor.matmul(gp[:, 0:1], expt[:, 0:128], t1[:], start=True, stop=True)
            nc.tensor.matmul(gp[:, 1:2], expt[:, 128:256], t1[:], start=True, stop=True)
            gs = pool.tile([128, 2], bf16, tag="gs")
            nc.vector.tensor_scalar_mul(gs[:], gp[:], czr[:])

            # ---- binarize mask-Toeplitz (X-0.5 > 0), 6 pieces, 2 engines ----
            mb = pool.tile([128, MW], bf16, tag="mb")
            for lo, hi, eng in (
                (0, 192, nc.vector),
                (192, 384, nc.vector),
                (384, 576, nc.vector),
                (576, 768, nc.gpsimd),
                (768, 960, nc.gpsimd),
                (960, MW, nc.vector),
            ):
                eng.tensor_scalar(
                    mb[:, lo:hi], mraw[:, lo:hi], 0.0, None, mybir.AluOpType.is_gt
                )

            # ---- conv: 9 accumulating banded-Toeplitz matmuls ----
            # op[r, b] += sum_u M_beta[u, r] * G[u, g] with b = beta - 2 + g
            for beta in range(1, NBETA + 1):
                mslice = mb[:, 128 * (beta - 1) : 128 * beta]
                if beta == 1:
                    o, g = op[:, 0:1], gs[:, 1:2]
                elif beta == NBETA:
                    o, g = op[:, NB - 1 : NB], gs[:, 0:1]
                else:
                    o, g = op[:, beta - 2 : beta], gs[:, 0:2]
                # middle matmuls skip the sim's bank-granular group check
                # (their slices don't cover the whole started region); the
                # last one keeps it so stop=True closes the group.
                nc.tensor.matmul(
                    o, mslice, g, start=False, stop=(beta == NBETA),
                    skip_group_check=(beta != NBETA),
                )

            # ---- add noise (fused with PSUM read), store ----
            outS = pool.tile([128, NB], f32, tag="outS")
            nc.vector.tensor_add(outS[:], op[:], nzT)
            nc.sync.dma_start(out_d[:], outS[:])

    nc.compile()
    return nc


def kernel(X, sigma, weight, noise):
    global _compiled
    import ml_dtypes

    from concourse.bass_utils import run_bass_kernel_spmd

    X = np.ascontiguousarray(np.asarray(X, dtype=np.float32)).reshape(1, T)
    sigma = np.ascontiguousarray(np.asarray(sigma, dtype=np.float32)).reshape(E)
    weight = np.ascontiguousarray(np.asarray(weight, dtype=np.float32)).reshape(1, E)
    noise = np.ascontiguousarray(np.asarray(noise, dtype=np.float32)).reshape(1, T)

    if _compiled is None:
        _compiled = _build()
    nc = _compiled

    # mask-Toeplitz expansion (layout + 0.5 shift; binarization on device):
    # M_beta[u, r] = Xpad[c*1024 + 128*beta + 126 - u + r] - 0.5
    Xpad = np.full(256 + T + 512, -0.5, dtype=np.float32)
    Xpad[256 : 256 + T] = X[0] - 0.5
    sw = np.lib.stride_tricks.sliding_window_view(Xpad, 128)
    u = np.arange(128)[:, None]
    betas = 128 * np.arange(1, NBETA + 1)[None, :]
    in_maps = []
    for c in range(N_CORES):
        idx = c * TJ + 126 + betas - u                      # [128, 9]
        mtoep = sw[idx].reshape(128, MW).astype(ml_dtypes.bfloat16)
        parcel = np.empty((128, NB + 2), dtype=np.float32)
        parcel[:, 0:NB] = noise[0, c * TJ : (c + 1) * TJ].reshape(NB, 128).T
        parcel[:, NB] = sigma
        parcel[:, NB + 1] = weight[0]
        in_maps.append({"parcel": parcel, "mtoep": mtoep})

    res = run_bass_kernel_spmd(nc, in_maps, core_ids=list(range(N_CORES)))
    out = np.empty((1, T), dtype=np.float32)
    for c in range(N_CORES):
        out[0, c * TJ : (c + 1) * TJ] = res.results[c]["out"].T.reshape(-1)
    return out


# revision 8
# speedup vs baseline: 1.8560x; 1.3746x over previous
"""Trainium2 Bass kernel for nn_CAGKE_1 (Gaussian-kernel embedding).

Math: reference computes, for mask m_i = 1[X_i > 0.5],
    out[j] = sum_e softmax(w)_e * sum_i m_i * (c/sigma_e) exp(-(j-i-1)^2/(2 sigma_e^2)) + noise_j
Both sums are linear, so the E=128 Gaussian channels collapse into one
combined kernel ghat(d) = sum_e softmax(w)_e * (c/sigma_e) exp(-d^2/(2 sigma_e^2))
BEFORE the convolution; 255 taps (|d| <= 127ish) are exact at f32 precision.

The conv is a banded-Toeplitz matmul whose Toeplitz factor lives in the MASK
operand: the host sends 9 blocks M_beta[u, r] = Xwin[128*beta + r - u + 126]
- 0.5 as bf16 (pure layout + affine shift; the sign of X-0.5 is exactly
preserved by bf16 rounding, so the device's >0 binarize reproduces X>0.5
bit-exactly). ghat is produced directly as a COLUMN G[u, g] = ghat(u + 128g
- 127) by two matmuls with the per-sigma exp table as the stationary
operand, so there is no DRAM round-trip and no PE transpose anywhere:

  out^T[r, b] = sum_{g=0,1} sum_u M_{b+2-g}[u, r] * G[u, g] + noise^T

  - softmax is computed max-free on a w COLUMN: Z is broadcast to all
    partitions by a matmul with an all-(1/c) stationary; 1/Z is folded into
    the PSUM->SBUF copy of G, so nothing softmax-related gates the exp table.
  - PSUM is zeroed by one early matmul with a zeros moving operand; the 9
    conv matmuls then pure-accumulate into column slices.
  - binarize is chunked into 6 small pieces split across DVE/GpSimd so no
    single piece can head-of-line-block the critical vector chain.
  - all DMAs ride hardware DGE (Scalar: parcel, Sync: mask chunks + store);
    the GpSimd software DGE is ~13 GB/s and stalls the whole core.
"""

import sys

import numpy as np

if "/opt/trn_rl_repo" not in sys.path:
    sys.path.insert(0, "/opt/trn_rl_repo")

T = 8192
E = 128
N_CORES = 8
TJ = T // N_CORES          # 1024 outputs per core
NB = TJ // 128             # 8 output blocks of 128
NBETA = 9                  # mask-Toeplitz blocks beta = 1..9
MW = NBETA * 128           # 1152 mask-Toeplitz columns
LK = 256                   # ghat taps, d = t - 127 for t in [0, 256)
INV_SQRT_2PI = 0.39894228

_compiled = None


def _build():
    import concourse.bacc as bacc
    import concourse.bass as bass
    import concourse.mybir as mybir
    import concourse.tile as tile

    f32 = mybir.dt.float32
    bf16 = mybir.dt.bfloat16
    nc = bacc.Bacc(num_devices=N_CORES, debug=False)

    parcel_d = nc.dram_tensor("parcel", [128, NB + 2], f32, kind="ExternalInput")
    mtoep_d = nc.dram_tensor("mtoep", [128, MW], bf16, kind="ExternalInput")
    out_d = nc.dram_tensor("out", [128, NB], f32, kind="ExternalOutput")

    with tile.TileContext(nc) as tc:
        with (
            tc.tile_pool(name="pool", bufs=1) as pool,
            tc.tile_pool(name="psum", bufs=1, space="PSUM") as psum,
        ):
            # ---- input loads, all on Sync-HWDGE: the tiny parcel MUST be
            # issued before the big mask transfer or its descriptors queue
            # behind 288KB in the DMA fabric (+4us observed) ----
            parcel = pool.tile([128, NB + 2], f32, tag="parcel")
            nc.sync.dma_start(parcel[:], parcel_d[:])
            mraw = pool.tile([128, MW], bf16, tag="mraw")
            nc.sync.dma_start(mraw[:, 0:576], mtoep_d[:, 0:576])
            nc.sync.dma_start(mraw[:, 576:MW], mtoep_d[:, 576:MW])
            nzT = parcel[:, 0:NB]
            sgT = parcel[:, NB : NB + 1]
            wT = parcel[:, NB + 1 : NB + 2]

            # ---- input-independent prep ----
            zeros8 = pool.tile([128, NB], f32, tag="zeros8")
            nc.vector.memset(zeros8[:], 0.0)
            onesInv = pool.tile([128, 128], f32, tag="onesInv")
            nc.vector.memset(onesInv[:], 1.0 / INV_SQRT_2PI)
            dlt = pool.tile([128, LK], f32, tag="dlt")
            nc.gpsimd.iota(
                dlt[:], pattern=[[1, LK]], base=-127, channel_multiplier=0,
                allow_small_or_imprecise_dtypes=True,
            )
            d2 = pool.tile([128, LK], f32, tag="d2")
            nc.gpsimd.tensor_mul(d2[:], dlt[:], dlt[:])

            # zero the output PSUM bank right away; conv matmuls accumulate.
            # (its bank's accumulation group stays open while zp/gp run in
            # their own banks, which hardware and sim both allow)
            op = psum.tile([128, NB], f32, tag="op")
            nc.tensor.matmul(op[:], onesInv[:], zeros8[:], start=True, stop=False)

            # ---- sigma column chain (starts as soon as parcel lands) ----
            s2 = pool.tile([128, 1], f32, tag="s2")
            nc.vector.tensor_mul(s2[:], sgT, sgT)
            nc.vector.tensor_scalar_mul(s2[:], s2[:], -2.0)
            invs = pool.tile([128, 1], f32, tag="invs")
            nc.vector.reciprocal(invs[:], s2[:])          # -1/(2 sigma^2)
            rs = pool.tile([128, 1], f32, tag="rs")
            nc.vector.reciprocal(rs[:], sgT)              # 1/sigma

            # ---- softmax pieces, max-free, all in column space ----
            exp_col = pool.tile([128, 1], f32, tag="exp_col")
            nc.scalar.activation(exp_col[:], wT, mybir.ActivationFunctionType.Exp)
            t1 = pool.tile([128, 1], f32, tag="t1")
            nc.vector.tensor_mul(t1[:], exp_col[:], rs[:])   # exp(w)/sigma
            zp = psum.tile([128, 1], f32, tag="zp")
            nc.tensor.matmul(zp[:], onesInv[:], exp_col[:], start=True, stop=True)
            czr = pool.tile([128, 1], f32, tag="czr")
            nc.vector.reciprocal(czr[:], zp[:])           # c/Z on every partition

            # ---- per-sigma exp table, e on partitions ----
            expt = pool.tile([128, LK], f32, tag="expt")
            nc.scalar.activation(
                expt[:], d2[:], mybir.ActivationFunctionType.Exp, scale=invs[:]
            )

            # ---- ghat as a column: G[u, g] = ghat(u + 128 g - 127) ----
            # (1/Z folded into the PSUM->SBUF copy, bf16 convert for the conv)
            gp = psum.tile([128, 2], f32, tag="gp")
            nc.tensor.matmul(gp[:, 0:1], expt[:, 0:128], t1[:], start=True, stop=True)
            nc.tensor.matmul(gp[:, 1:2], expt[:, 128:256], t1[:], start=True, stop=True)
            gs = pool.tile([128, 2], bf16, tag="gs")
            nc.vector.tensor_scalar_mul(gs[:], gp[:], czr[:])

            # ---- binarize mask-Toeplitz (X-0.5 > 0), 6 pieces, 2 engines ----
            mb = pool.tile([128, MW], bf16, tag="mb")
            # all pieces on DVE: GpSimd runs tensor_scalar at ~16ns/col and
            # would gate the tail conv matmuls by ~3us per piece
            for lo, hi in ((0, 192), (192, 384), (384, 576),
                           (576, 768), (768, 960), (960, MW)):
                nc.vector.tensor_scalar(
                    mb[:, lo:hi], mraw[:, lo:hi], 0.0, None, mybir.AluOpType.is_gt
                )

            # ---- conv: 9 accumulating banded-Toeplitz matmuls ----
            # op[r, b] += sum_u M_beta[u, r] * G[u, g] with b = beta - 2 + g
            for beta in range(1, NBETA + 1):
                mslice = mb[:, 128 * (beta - 1) : 128 * beta]
                if beta == 1:
                    o, g = op[:, 0:1], gs[:, 1:2]
                elif beta == NBETA:
                    o, g = op[:, NB - 1 : NB], gs[:, 0:1]
                else:
                    o, g = op[:, beta - 2 : beta], gs[:, 0:2]
                # middle matmuls skip the sim's bank-granular group check
                # (their slices don't cover the whole started region); the
                # last one keeps it so stop=True closes the group.
                nc.tensor.matmul(
                    o, mslice, g, start=False, stop=(beta == NBETA),
                    skip_group_check=(beta != NBETA),
                )

            # ---- add noise (fused with PSUM read), store ----
            outS = pool.tile([128, NB], f32, tag="outS")
            nc.vector.tensor_add(outS[:], op[:], nzT)
            nc.sync.dma_start(out_d[:], outS[:])

    nc.compile()
    return nc


def kernel(X, sigma, weight, noise):
    global _compiled
    import ml_dtypes

    from concourse.bass_utils import run_bass_kernel_spmd

    X = np.ascontiguousarray(np.asarray(X, dtype=np.float32)).reshape(1, T)
    sigma = np.ascontiguousarray(np.asarray(sigma, dtype=np.float32)).reshape(E)
    weight = np.ascontiguousarray(np.asarray(weight, dtype=np.float32)).reshape(1, E)
    noise = np.ascontiguousarray(np.asarray(noise, dtype=np.float32)).reshape(1, T)

    if _compiled is None:
        _compiled = _build()
    nc = _compiled

    # mask-Toeplitz expansion (layout + 0.5 shift; binarization on device):
    # M_beta[u, r] = Xpad[c*1024 + 128*beta + 126 - u + r] - 0.5
    Xpad = np.full(256 + T + 512, -0.5, dtype=np.float32)
    Xpad[256 : 256 + T] = X[0] - 0.5
    sw = np.lib.stride_tricks.sliding_window_view(Xpad, 128)
    u = np.arange(128)[:, None]
    betas = 128 * np.arange(1, NBETA + 1)[None, :]
    in_maps = []
    for c in range(N_CORES):
        idx = c * TJ + 126 + betas - u                      # [128, 9]
        mtoep = sw[idx].reshape(128, MW).astype(ml_dtypes.bfloat16)
        parcel = np.empty((128, NB + 2), dtype=np.float32)
        parcel[:, 0:NB] = noise[0, c * TJ : (c + 1) * TJ].reshape(NB, 128).T
        parcel[:, NB] = sigma
        parcel[:, NB + 1] = weight[0]
        in_maps.append({"parcel": parcel, "mtoep": mtoep})

    res = run_bass_kernel_spmd(nc, in_maps, core_ids=list(range(N_CORES)))
    out = np.empty((1, T), dtype=np.float32)
    for c in range(N_CORES):
        out[0, c * TJ : (c + 1) * TJ] = res.results[c]["out"].T.reshape(-1)
    return out


# revision 11
# speedup vs baseline: 1.9276x; 1.0386x over previous
"""Trainium2 Bass kernel for nn_CAGKE_1 (Gaussian-kernel embedding).

Math: reference computes, for mask m_i = 1[X_i > 0.5],
    out[j] = sum_e softmax(w)_e * sum_i m_i * (c/sigma_e) exp(-(j-i-1)^2/(2 sigma_e^2)) + noise_j
Both sums are linear, so the E=128 Gaussian channels collapse into one
combined kernel ghat(d) = sum_e softmax(w)_e * (c/sigma_e) exp(-d^2/(2 sigma_e^2))
BEFORE the convolution; 255 taps (|d| <= 127ish) cover it exactly.

The conv is a banded-Toeplitz matmul whose Toeplitz factor lives in the MASK
operand: the host sends 9 blocks M_beta[u, r] = Xwin[128*beta + r - u + 126]
- 0.5 as fp8e4 (pure layout + affine shift; the sign of X-0.5 survives fp8
rounding except within 2^-10 of the threshold, ~1 element per core window).
The device binarizes with >0. ghat is produced directly as a COLUMN
G[u, g] = ghat(u + 128g - 127) by two bf16 matmuls with the per-sigma exp
table as the stationary operand, so there is no DRAM round-trip and no PE
transpose anywhere:

  out^T[r, b] = sum_{g=0,1} sum_u M_{b+2-g}[u, r] * G[u, g] + noise^T

  - softmax is computed max-free on a w COLUMN: Z is broadcast to all
    partitions by a matmul with an all-(1/c) stationary; 1/Z is folded into
    the PSUM->SBUF copy of G, so nothing softmax-related gates the exp table.
  - PSUM is zeroed by one early matmul with a zeros moving operand; the 9
    conv matmuls then pure-accumulate into column slices.
  - binarize is chunked into 6 DVE pieces, data-dependent on t1 (via a
    zeroed threshold tile) so the tile scheduler cannot reorder them ahead
    of the latency-critical softmax/sigma chain.
  - the tiny parcel DMA is issued before the mask transfer on the same
    Sync-HWDGE path or its descriptors queue behind 144KB in the DMA fabric.
  - GpSimd only runs iota/square: its software DGE (~13 GB/s) and its
    tensor_scalar (~16ns/col) both stall the core if used for real work.
"""

import sys

import numpy as np

if "/opt/trn_rl_repo" not in sys.path:
    sys.path.insert(0, "/opt/trn_rl_repo")

T = 8192
E = 128
N_CORES = 8
TJ = T // N_CORES          # 1024 outputs per core
NB = TJ // 128             # 8 output blocks of 128
NBETA = 9                  # mask-Toeplitz blocks beta = 1..9
MW = NBETA * 128           # 1152 mask-Toeplitz columns
LK = 256                   # ghat taps, d = t - 127 for t in [0, 256)
INV_SQRT_2PI = 0.39894228

_compiled = None


def _build():
    import concourse.bacc as bacc
    import concourse.bass as bass
    import concourse.mybir as mybir
    import concourse.tile as tile

    f32 = mybir.dt.float32
    bf16 = mybir.dt.bfloat16
    fp8 = mybir.dt.float8e4
    nc = bacc.Bacc(num_devices=N_CORES, debug=False)

    parcel_d = nc.dram_tensor("parcel", [128, NB + 2], f32, kind="ExternalInput")
    mtoep_d = nc.dram_tensor("mtoep", [128, MW], fp8, kind="ExternalInput")
    out_d = nc.dram_tensor("out", [128, NB], f32, kind="ExternalOutput")

    with tile.TileContext(nc) as tc:
        with (
            tc.tile_pool(name="pool", bufs=1) as pool,
            tc.tile_pool(name="psum", bufs=1, space="PSUM") as psum,
        ):
            # ---- input loads, parcel first ----
            parcel = pool.tile([128, NB + 2], f32, tag="parcel")
            nc.sync.dma_start(parcel[:], parcel_d[:])
            mraw = pool.tile([128, MW], fp8, tag="mraw")
            nc.sync.dma_start(mraw[:, 0:576], mtoep_d[:, 0:576])
            nc.sync.dma_start(mraw[:, 576:MW], mtoep_d[:, 576:MW])
            nzT = parcel[:, 0:NB]
            sgT = parcel[:, NB : NB + 1]
            wT = parcel[:, NB + 1 : NB + 2]

            # ---- input-independent prep ----
            zeros8 = pool.tile([128, NB], f32, tag="zeros8")
            nc.vector.memset(zeros8[:], 0.0)
            onesInv = pool.tile([128, 128], f32, tag="onesInv")
            nc.vector.memset(onesInv[:], 1.0 / INV_SQRT_2PI)
            dlt = pool.tile([128, LK], f32, tag="dlt")
            nc.gpsimd.iota(
                dlt[:], pattern=[[1, LK]], base=-127, channel_multiplier=0,
                allow_small_or_imprecise_dtypes=True,
            )
            d2 = pool.tile([128, LK], f32, tag="d2")
            nc.gpsimd.tensor_mul(d2[:], dlt[:], dlt[:])

            # zero the output PSUM bank right away; conv matmuls accumulate.
            op = psum.tile([128, NB], f32, tag="op")
            nc.tensor.matmul(op[:], onesInv[:], zeros8[:], start=True, stop=False)

            # ---- sigma column chain: invs = -1/(2 sigma^2) in 2 ops ----
            s2 = pool.tile([128, 1], f32, tag="s2")
            nc.vector.tensor_scalar(
                s2[:], sgT, sgT, -2.0, mybir.AluOpType.mult, mybir.AluOpType.mult
            )
            invs = pool.tile([128, 1], f32, tag="invs")
            nc.vector.reciprocal(invs[:], s2[:])
            rs = pool.tile([128, 1], f32, tag="rs")
            nc.vector.reciprocal(rs[:], sgT)              # 1/sigma

            # ---- softmax pieces, max-free, all in column space ----
            exp_col = pool.tile([128, 1], f32, tag="exp_col")
            nc.scalar.activation(exp_col[:], wT, mybir.ActivationFunctionType.Exp)
            t1 = pool.tile([128, 1], bf16, tag="t1")
            nc.vector.tensor_mul(t1[:], exp_col[:], rs[:])   # exp(w)/sigma
            zp = psum.tile([128, 1], f32, tag="zp")
            nc.tensor.matmul(zp[:], onesInv[:], exp_col[:], start=True, stop=True)
            czr = pool.tile([128, 1], f32, tag="czr")
            nc.vector.reciprocal(czr[:], zp[:])           # c/Z on every partition

            # binarize threshold = 0, derived from t1 so the scheduler keeps
            # the binarize pieces behind the critical chain
            thr = pool.tile([128, 1], f32, tag="thr")
            nc.vector.tensor_scalar_mul(thr[:], t1[:], 0.0)

            # ---- per-sigma exp table, e on partitions, bf16 for fast LDW ----
            expt = pool.tile([128, LK], bf16, tag="expt")
            nc.scalar.activation(
                expt[:], d2[:], mybir.ActivationFunctionType.Exp, scale=invs[:]
            )

            # ---- ghat as a column: G[u, g] = ghat(u + 128 g - 127) ----
            gp = psum.tile([128, 2], f32, tag="gp")
            nc.tensor.matmul(gp[:, 0:1], expt[:, 0:128], t1[:], start=True, stop=True)
            nc.tensor.matmul(gp[:, 1:2], expt[:, 128:256], t1[:], start=True, stop=True)
            gs = pool.tile([128, 2], bf16, tag="gs")
            nc.vector.tensor_scalar_mul(gs[:], gp[:], czr[:])

            # ---- binarize mask-Toeplitz (X-0.5 > 0), 6 DVE pieces ----
            mb = pool.tile([128, MW], bf16, tag="mb")
            for lo, hi in ((0, 192), (192, 384), (384, 576),
                           (576, 768), (768, 960), (960, MW)):
                nc.vector.tensor_scalar(
                    mb[:, lo:hi], mraw[:, lo:hi], thr[:], None, mybir.AluOpType.is_gt
                )

            # ---- conv: 9 accumulating banded-Toeplitz matmuls ----
            # op[r, b] += sum_u M_beta[u, r] * G[u, g] with b = beta - 2 + g
            for beta in range(1, NBETA + 1):
                mslice = mb[:, 128 * (beta - 1) : 128 * beta]
                if beta == 1:
                    o, g = op[:, 0:1], gs[:, 1:2]
                elif beta == NBETA:
                    o, g = op[:, NB - 1 : NB], gs[:, 0:1]
                else:
                    o, g = op[:, beta - 2 : beta], gs[:, 0:2]
                # middle matmuls skip the sim's bank-granular group check
                # (their slices don't cover the whole started region); the
                # last one keeps it so stop=True closes the group.
                nc.tensor.matmul(
                    o, mslice, g, start=False, stop=(beta == NBETA),
                    skip_group_check=(beta != NBETA),
                )

            # ---- add noise (fused with PSUM read), store ----
            outS = pool.tile([128, NB], f32, tag="outS")
            nc.vector.tensor_add(outS[:], op[:], nzT)
            nc.sync.dma_start(out_d[:], outS[:])

    nc.compile()
    return nc


def kernel(X, sigma, weight, noise):
    global _compiled
    import ml_dtypes

    from concourse.bass_utils import run_bass_kernel_spmd

    X = np.ascontiguousarray(np.asarray(X, dtype=np.float32)).reshape(1, T)
    sigma = np.ascontiguousarray(np.asarray(sigma, dtype=np.float32)).reshape(E)
    weight = np.ascontiguousarray(np.asarray(weight, dtype=np.float32)).reshape(1, E)
    noise = np.ascontiguousarray(np.asarray(noise, dtype=np.float32)).reshape(1, T)

    if _compiled is None:
        _compiled = _build()
    nc = _compiled

    # mask-Toeplitz expansion (layout + affine shift; binarize on device):
    # M_beta[u, r] = 64*(Xpad[c*1024 + 128*beta + 126 - u + r] - 0.5)
    # The 64x scale shrinks fp8's round-to-zero dead zone around the
    # threshold from 2^-10 to 2^-16 so the sign (= the mask bit) survives.
    Xpad = np.full(256 + T + 512, -32.0, dtype=np.float32)
    Xpad[256 : 256 + T] = 64.0 * (X[0] - 0.5)
    sw = np.lib.stride_tricks.sliding_window_view(Xpad, 128)
    u = np.arange(128)[:, None]
    betas = 128 * np.arange(1, NBETA + 1)[None, :]
    in_maps = []
    for c in range(N_CORES):
        idx = c * TJ + 126 + betas - u                      # [128, 9]
        mtoep = sw[idx].reshape(128, MW).astype(ml_dtypes.float8_e4m3)
        parcel = np.empty((128, NB + 2), dtype=np.float32)
        parcel[:, 0:NB] = noise[0, c * TJ : (c + 1) * TJ].reshape(NB, 128).T
        parcel[:, NB] = sigma
        parcel[:, NB + 1] = weight[0]
        in_maps.append({"parcel": parcel, "mtoep": mtoep})

    res = run_bass_kernel_spmd(nc, in_maps, core_ids=list(range(N_CORES)))
    out = np.empty((1, T), dtype=np.float32)
    for c in range(N_CORES):
        out[0, c * TJ : (c + 1) * TJ] = res.results[c]["out"].T.reshape(-1)
    return out


# revision 13
# speedup vs baseline: 1.9417x; 1.0073x over previous
"""Trainium2 Bass kernel for nn_CAGKE_1 (Gaussian-kernel embedding).

Math: reference computes, for mask m_i = 1[X_i > 0.5],
    out[j] = sum_e softmax(w)_e * sum_i m_i * (c/sigma_e) exp(-(j-i-1)^2/(2 sigma_e^2)) + noise_j
Both sums are linear, so the E=128 Gaussian channels collapse into one
combined kernel ghat(d) = sum_e softmax(w)_e * (c/sigma_e) exp(-d^2/(2 sigma_e^2))
BEFORE the convolution; 255 taps (|d| <= 127ish) cover it exactly.

The conv is a banded-Toeplitz matmul whose Toeplitz factor lives in the MASK
operand: the host sends 9 blocks M_beta[u, r] = Xwin[128*beta + r - u + 126]
- 0.5 as fp8e4 (pure layout + affine shift; the sign of X-0.5 survives fp8
rounding except within 2^-10 of the threshold, ~1 element per core window).
The device binarizes with >0. ghat is produced directly as a COLUMN
G[u, g] = ghat(u + 128g - 127) by two bf16 matmuls with the per-sigma exp
table as the stationary operand, so there is no DRAM round-trip and no PE
transpose anywhere:

  out^T[r, b] = sum_{g=0,1} sum_u M_{b+2-g}[u, r] * G[u, g] + noise^T

  - softmax is computed max-free on a w COLUMN: Z is broadcast to all
    partitions by a matmul with an all-(1/c) stationary; 1/Z is folded into
    the PSUM->SBUF copy of G, so nothing softmax-related gates the exp table.
  - PSUM is zeroed by one early matmul with a zeros moving operand; the 9
    conv matmuls then pure-accumulate into column slices.
  - binarize is chunked into 6 DVE pieces, data-dependent on t1 (via a
    zeroed threshold tile) so the tile scheduler cannot reorder them ahead
    of the latency-critical softmax/sigma chain.
  - the tiny parcel DMA is issued before the mask transfer on the same
    Sync-HWDGE path or its descriptors queue behind 144KB in the DMA fabric.
  - GpSimd only runs iota/square: its software DGE (~13 GB/s) and its
    tensor_scalar (~16ns/col) both stall the core if used for real work.
"""

import sys

import numpy as np

if "/opt/trn_rl_repo" not in sys.path:
    sys.path.insert(0, "/opt/trn_rl_repo")

T = 8192
E = 128
N_CORES = 8
TJ = T // N_CORES          # 1024 outputs per core
NB = TJ // 128             # 8 output blocks of 128
NBETA = 9                  # mask-Toeplitz blocks beta = 1..9
MW = NBETA * 128           # 1152 mask-Toeplitz columns
LK = 256                   # ghat taps, d = t - 127 for t in [0, 256)
INV_SQRT_2PI = 0.39894228

_compiled = None


def _build():
    import concourse.bacc as bacc
    import concourse.bass as bass
    import concourse.mybir as mybir
    import concourse.tile as tile

    f32 = mybir.dt.float32
    bf16 = mybir.dt.bfloat16
    nc = bacc.Bacc(num_devices=N_CORES, debug=False)

    parcel_d = nc.dram_tensor("parcel", [128, NB + 2], f32, kind="ExternalInput")
    mtoep_d = nc.dram_tensor("mtoep", [128, MW], bf16, kind="ExternalInput")
    out_d = nc.dram_tensor("out", [128, NB], f32, kind="ExternalOutput")

    with tile.TileContext(nc) as tc:
        with (
            tc.tile_pool(name="pool", bufs=1) as pool,
            tc.tile_pool(name="psum", bufs=1, space="PSUM") as psum,
        ):
            # ---- input loads, parcel first ----
            parcel = pool.tile([128, NB + 2], f32, tag="parcel")
            nc.sync.dma_start(parcel[:], parcel_d[:])
            mraw = pool.tile([128, MW], bf16, tag="mraw")
            nc.sync.dma_start(mraw[:], mtoep_d[:])
            nzT = parcel[:, 0:NB]
            sgT = parcel[:, NB : NB + 1]
            wT = parcel[:, NB + 1 : NB + 2]

            # ---- input-independent prep ----
            zeros8 = pool.tile([128, NB], f32, tag="zeros8")
            nc.vector.memset(zeros8[:], 0.0)
            onesInv = pool.tile([128, 128], f32, tag="onesInv")
            nc.vector.memset(onesInv[:], 1.0 / INV_SQRT_2PI)
            dlt = pool.tile([128, LK], f32, tag="dlt")
            nc.gpsimd.iota(
                dlt[:], pattern=[[1, LK]], base=-127, channel_multiplier=0,
                allow_small_or_imprecise_dtypes=True,
            )
            d2 = pool.tile([128, LK], f32, tag="d2")
            nc.gpsimd.tensor_mul(d2[:], dlt[:], dlt[:])

            # zero the output PSUM bank right away; conv matmuls accumulate.
            op = psum.tile([128, NB], f32, tag="op")
            nc.tensor.matmul(op[:], onesInv[:], zeros8[:], start=True, stop=False)

            # ---- sigma column chain: invs = -1/(2 sigma^2) in 2 ops ----
            s2 = pool.tile([128, 1], f32, tag="s2")
            nc.vector.tensor_scalar(
                s2[:], sgT, sgT, -2.0, mybir.AluOpType.mult, mybir.AluOpType.mult
            )
            invs = pool.tile([128, 1], f32, tag="invs")
            nc.vector.reciprocal(invs[:], s2[:])
            rs = pool.tile([128, 1], f32, tag="rs")
            nc.vector.reciprocal(rs[:], sgT)              # 1/sigma

            # ---- softmax pieces, max-free, all in column space ----
            exp_col = pool.tile([128, 1], f32, tag="exp_col")
            nc.scalar.activation(exp_col[:], wT, mybir.ActivationFunctionType.Exp)
            t1 = pool.tile([128, 1], bf16, tag="t1")
            nc.vector.tensor_mul(t1[:], exp_col[:], rs[:])   # exp(w)/sigma
            zp = psum.tile([128, 1], f32, tag="zp")
            nc.tensor.matmul(zp[:], onesInv[:], exp_col[:], start=True, stop=True)
            czr = pool.tile([128, 1], f32, tag="czr")
            nc.vector.reciprocal(czr[:], zp[:])           # c/Z on every partition

            # binarize threshold = 0, derived from exp_col so the scheduler
            # keeps the binarize pieces behind the critical-chain start
            thr = pool.tile([128, 1], f32, tag="thr")
            nc.vector.tensor_scalar_mul(thr[:], exp_col[:], 0.0)

            # ---- per-sigma exp table, e on partitions, bf16 for fast LDW ----
            expt = pool.tile([128, LK], bf16, tag="expt")
            nc.scalar.activation(
                expt[:], d2[:], mybir.ActivationFunctionType.Exp, scale=invs[:]
            )

            # ---- ghat as a column: G[u, g] = ghat(u + 128 g - 127) ----
            gp = psum.tile([128, 2], f32, tag="gp")
            nc.tensor.matmul(gp[:, 0:1], expt[:, 0:128], t1[:], start=True, stop=True)
            nc.tensor.matmul(gp[:, 1:2], expt[:, 128:256], t1[:], start=True, stop=True)
            gs = pool.tile([128, 2], bf16, tag="gs")
            nc.vector.tensor_scalar_mul(gs[:], gp[:], czr[:])

            # ---- binarize mask-Toeplitz (X-0.5 > 0), 6 DVE pieces ----
            mb = pool.tile([128, MW], bf16, tag="mb")
            for lo, hi in ((0, 192), (192, 384), (384, 576),
                           (576, 768), (768, 960), (960, MW)):
                nc.vector.tensor_scalar(
                    mb[:, lo:hi], mraw[:, lo:hi], thr[:], None, mybir.AluOpType.is_gt
                )

            # ---- conv: 9 accumulating banded-Toeplitz matmuls ----
            # op[r, b] += sum_u M_beta[u, r] * G[u, g] with b = beta - 2 + g
            for beta in range(1, NBETA + 1):
                mslice = mb[:, 128 * (beta - 1) : 128 * beta]
                if beta == 1:
                    o, g = op[:, 0:1], gs[:, 1:2]
                elif beta == NBETA:
                    o, g = op[:, NB - 1 : NB], gs[:, 0:1]
                else:
                    o, g = op[:, beta - 2 : beta], gs[:, 0:2]
                # middle matmuls skip the sim's bank-granular group check
                # (their slices don't cover the whole started region); the
                # last one keeps it so stop=True closes the group.
                nc.tensor.matmul(
                    o, mslice, g, start=False, stop=(beta == NBETA),
                    skip_group_check=(beta != NBETA),
                )

            # ---- add noise (fused with PSUM read), store ----
            outS = pool.tile([128, NB], f32, tag="outS")
            nc.vector.tensor_add(outS[:], op[:], nzT)
            nc.sync.dma_start(out_d[:], outS[:])

    nc.compile()
    return nc


def kernel(X, sigma, weight, noise):
    global _compiled
    import ml_dtypes

    from concourse.bass_utils import run_bass_kernel_spmd

    X = np.ascontiguousarray(np.asarray(X, dtype=np.float32)).reshape(1, T)
    sigma = np.ascontiguousarray(np.asarray(sigma, dtype=np.float32)).reshape(E)
    weight = np.ascontiguousarray(np.asarray(weight, dtype=np.float32)).reshape(1, E)
    noise = np.ascontiguousarray(np.asarray(noise, dtype=np.float32)).reshape(1, T)

    if _compiled is None:
        _compiled = _build()
    nc = _compiled

    # mask-Toeplitz expansion (layout + affine shift; binarize on device):
    # M_beta[u, r] = Xpad[c*1024 + 128*beta + 126 - u + r] - 0.5
    # (bf16 = truncated f32: rounding never crosses zero, sign is exact)
    Xpad = np.full(256 + T + 512, -0.5, dtype=np.float32)
    Xpad[256 : 256 + T] = X[0] - 0.5
    sw = np.lib.stride_tricks.sliding_window_view(Xpad, 128)
    u = np.arange(128)[:, None]
    betas = 128 * np.arange(1, NBETA + 1)[None, :]
    in_maps = []
    for c in range(N_CORES):
        idx = c * TJ + 126 + betas - u                      # [128, 9]
        mtoep = sw[idx].reshape(128, MW).astype(ml_dtypes.bfloat16)
        parcel = np.empty((128, NB + 2), dtype=np.float32)
        parcel[:, 0:NB] = noise[0, c * TJ : (c + 1) * TJ].reshape(NB, 128).T
        parcel[:, NB] = sigma
        parcel[:, NB + 1] = weight[0]
        in_maps.append({"parcel": parcel, "mtoep": mtoep})

    res = run_bass_kernel_spmd(nc, in_maps, core_ids=list(range(N_CORES)))
    out = np.empty((1, T), dtype=np.float32)
    for c in range(N_CORES):
        out[0, c * TJ : (c + 1) * TJ] = res.results[c]["out"].T.reshape(-1)
    return out


# revision 14
# speedup vs baseline: 2.0037x; 1.0319x over previous
"""Trainium2 Bass kernel for nn_CAGKE_1 (Gaussian-kernel embedding).

Math: reference computes, for mask m_i = 1[X_i > 0.5],
    out[j] = sum_e softmax(w)_e * sum_i m_i * (c/sigma_e) exp(-(j-i-1)^2/(2 sigma_e^2)) + noise_j
Both sums are linear, so the E=128 Gaussian channels collapse into one
combined kernel ghat(d) = sum_e softmax(w)_e * (c/sigma_e) exp(-d^2/(2 sigma_e^2))
BEFORE the convolution; 255 taps (|d| <= 127ish) cover it exactly.

The conv is a banded-Toeplitz matmul whose Toeplitz factor lives in the MASK
operand: the host sends 9 blocks M_beta[u, r] = Xwin[128*beta + r - u + 126]
- 0.5 as fp8e4 (pure layout + affine shift; the sign of X-0.5 survives fp8
rounding except within 2^-10 of the threshold, ~1 element per core window).
The device binarizes with >0. ghat is produced directly as a COLUMN
G[u, g] = ghat(u + 128g - 127) by two bf16 matmuls with the per-sigma exp
table as the stationary operand, so there is no DRAM round-trip and no PE
transpose anywhere:

  out^T[r, b] = sum_{g=0,1} sum_u M_{b+2-g}[u, r] * G[u, g] + noise^T

  - softmax is computed max-free on a w COLUMN: Z is broadcast to all
    partitions by a matmul with an all-(1/c) stationary; 1/Z is folded into
    the PSUM->SBUF copy of G, so nothing softmax-related gates the exp table.
  - PSUM is zeroed by one early matmul with a zeros moving operand; the 9
    conv matmuls then pure-accumulate into column slices.
  - binarize is chunked into 6 DVE pieces, data-dependent on t1 (via a
    zeroed threshold tile) so the tile scheduler cannot reorder them ahead
    of the latency-critical softmax/sigma chain.
  - the tiny parcel DMA is issued before the mask transfer on the same
    Sync-HWDGE path or its descriptors queue behind 144KB in the DMA fabric.
  - GpSimd only runs iota/square: its software DGE (~13 GB/s) and its
    tensor_scalar (~16ns/col) both stall the core if used for real work.
"""

import sys

import numpy as np

if "/opt/trn_rl_repo" not in sys.path:
    sys.path.insert(0, "/opt/trn_rl_repo")

T = 8192
E = 128
N_CORES = 8
TJ = T // N_CORES          # 1024 outputs per core
NB = TJ // 128             # 8 output blocks of 128
NBETA = 9                  # mask-Toeplitz blocks beta = 1..9
MW = NBETA * 128           # 1152 mask-Toeplitz columns
LK = 256                   # ghat taps, d = t - 127 for t in [0, 256)
INV_SQRT_2PI = 0.39894228

_compiled = None


def _build():
    import concourse.bacc as bacc
    import concourse.bass as bass
    import concourse.mybir as mybir
    import concourse.tile as tile

    f32 = mybir.dt.float32
    bf16 = mybir.dt.bfloat16
    fp8 = mybir.dt.float8e4
    nc = bacc.Bacc(num_devices=N_CORES, debug=False)

    parcel_d = nc.dram_tensor("parcel", [128, NB + 2], f32, kind="ExternalInput")
    mtoep_d = nc.dram_tensor("mtoep", [128, MW], fp8, kind="ExternalInput")
    out_d = nc.dram_tensor("out", [128, NB], f32, kind="ExternalOutput")

    with tile.TileContext(nc) as tc:
        with (
            tc.tile_pool(name="pool", bufs=1) as pool,
            tc.tile_pool(name="psum", bufs=1, space="PSUM") as psum,
        ):
            # ---- input loads, parcel first ----
            parcel = pool.tile([128, NB + 2], f32, tag="parcel")
            nc.sync.dma_start(parcel[:], parcel_d[:])
            mraw = pool.tile([128, MW], fp8, tag="mraw")
            nc.sync.dma_start(mraw[:], mtoep_d[:])
            nzT = parcel[:, 0:NB]
            sgT = parcel[:, NB : NB + 1]
            wT = parcel[:, NB + 1 : NB + 2]

            # ---- input-independent prep ----
            zeros8 = pool.tile([128, NB], f32, tag="zeros8")
            nc.vector.memset(zeros8[:], 0.0)
            onesInv = pool.tile([128, 128], f32, tag="onesInv")
            nc.vector.memset(onesInv[:], 1.0 / INV_SQRT_2PI)
            dlt = pool.tile([128, LK], f32, tag="dlt")
            nc.gpsimd.iota(
                dlt[:], pattern=[[1, LK]], base=-127, channel_multiplier=0,
                allow_small_or_imprecise_dtypes=True,
            )
            d2 = pool.tile([128, LK], f32, tag="d2")
            nc.gpsimd.tensor_mul(d2[:], dlt[:], dlt[:])

            # zero the output PSUM bank right away; conv matmuls accumulate.
            op = psum.tile([128, NB], f32, tag="op")
            nc.tensor.matmul(op[:], onesInv[:], zeros8[:], start=True, stop=False)

            # ---- sigma column chain: invs = -1/(2 sigma^2) in 2 ops ----
            s2 = pool.tile([128, 1], f32, tag="s2")
            nc.vector.tensor_scalar(
                s2[:], sgT, sgT, -2.0, mybir.AluOpType.mult, mybir.AluOpType.mult
            )
            invs = pool.tile([128, 1], f32, tag="invs")
            nc.vector.reciprocal(invs[:], s2[:])
            rs = pool.tile([128, 1], f32, tag="rs")
            nc.vector.reciprocal(rs[:], sgT)              # 1/sigma

            # ---- softmax pieces, max-free, all in column space ----
            exp_col = pool.tile([128, 1], f32, tag="exp_col")
            nc.scalar.activation(exp_col[:], wT, mybir.ActivationFunctionType.Exp)
            t1 = pool.tile([128, 1], bf16, tag="t1")
            nc.vector.tensor_mul(t1[:], exp_col[:], rs[:])   # exp(w)/sigma
            zp = psum.tile([128, 1], f32, tag="zp")
            nc.tensor.matmul(zp[:], onesInv[:], exp_col[:], start=True, stop=True)
            czr = pool.tile([128, 1], f32, tag="czr")
            nc.vector.reciprocal(czr[:], zp[:])           # c/Z on every partition

            # binarize threshold = 0, derived from exp_col so the scheduler
            # keeps the binarize pieces behind the critical-chain start
            thr = pool.tile([128, 1], f32, tag="thr")
            nc.vector.tensor_scalar_mul(thr[:], exp_col[:], 0.0)

            # ---- per-sigma exp table, e on partitions, bf16 for fast LDW ----
            expt = pool.tile([128, LK], bf16, tag="expt")
            nc.scalar.activation(
                expt[:, 0:128], d2[:, 0:128],
                mybir.ActivationFunctionType.Exp, scale=invs[:]
            )
            nc.scalar.activation(
                expt[:, 128:LK], d2[:, 128:LK],
                mybir.ActivationFunctionType.Exp, scale=invs[:]
            )

            # ---- ghat as a column: G[u, g] = ghat(u + 128 g - 127) ----
            gp = psum.tile([128, 2], f32, tag="gp")
            nc.tensor.matmul(gp[:, 0:1], expt[:, 0:128], t1[:], start=True, stop=True)
            nc.tensor.matmul(gp[:, 1:2], expt[:, 128:256], t1[:], start=True, stop=True)
            gs = pool.tile([128, 2], bf16, tag="gs")
            nc.vector.tensor_scalar_mul(gs[:], gp[:], czr[:])

            # ---- binarize mask-Toeplitz (X-0.5 > 0), 6 DVE pieces ----
            mb = pool.tile([128, MW], bf16, tag="mb")
            for lo, hi in ((0, 384), (384, 768), (768, MW)):
                nc.vector.tensor_scalar(
                    mb[:, lo:hi], mraw[:, lo:hi], thr[:], None, mybir.AluOpType.is_gt
                )

            # ---- conv: 9 accumulating banded-Toeplitz matmuls ----
            # op[r, b] += sum_u M_beta[u, r] * G[u, g] with b = beta - 2 + g
            for beta in range(1, NBETA + 1):
                mslice = mb[:, 128 * (beta - 1) : 128 * beta]
                if beta == 1:
                    o, g = op[:, 0:1], gs[:, 1:2]
                elif beta == NBETA:
                    o, g = op[:, NB - 1 : NB], gs[:, 0:1]
                else:
                    o, g = op[:, beta - 2 : beta], gs[:, 0:2]
                # middle matmuls skip the sim's bank-granular group check
                # (their slices don't cover the whole started region); the
                # last one keeps it so stop=True closes the group.
                nc.tensor.matmul(
                    o, mslice, g, start=False, stop=(beta == NBETA),
                    skip_group_check=(beta != NBETA),
                )

            # ---- add noise (fused with PSUM read), store ----
            outS = pool.tile([128, NB], f32, tag="outS")
            nc.vector.tensor_add(outS[:], op[:], nzT)
            nc.sync.dma_start(out_d[:], outS[:])

    nc.compile()
    return nc


def kernel(X, sigma, weight, noise):
    global _compiled
    import ml_dtypes

    from concourse.bass_utils import run_bass_kernel_spmd

    X = np.ascontiguousarray(np.asarray(X, dtype=np.float32)).reshape(1, T)
    sigma = np.ascontiguousarray(np.asarray(sigma, dtype=np.float32)).reshape(E)
    weight = np.ascontiguousarray(np.asarray(weight, dtype=np.float32)).reshape(1, E)
    noise = np.ascontiguousarray(np.asarray(noise, dtype=np.float32)).reshape(1, T)

    if _compiled is None:
        _compiled = _build()
    nc = _compiled

    # mask-Toeplitz expansion (layout + affine shift; binarize on device):
    # M_beta[u, r] = 64*(Xpad[c*1024 + 128*beta + 126 - u + r] - 0.5)
    # The 64x scale shrinks fp8's round-to-zero dead zone around the
    # threshold from 2^-10 to 2^-16 so the sign (= the mask bit) survives.
    Xpad = np.full(256 + T + 512, -32.0, dtype=np.float32)
    Xpad[256 : 256 + T] = 64.0 * (X[0] - 0.5)
    sw = np.lib.stride_tricks.sliding_window_view(Xpad, 128)
    u = np.arange(128)[:, None]
    betas = 128 * np.arange(1, NBETA + 1)[None, :]
    in_maps = []
    for c in range(N_CORES):
        idx = c * TJ + 126 + betas - u                      # [128, 9]
        mtoep = sw[idx].reshape(128, MW).astype(ml_dtypes.float8_e4m3)
        parcel = np.empty((128, NB + 2), dtype=np.float32)
        parcel[:, 0:NB] = noise[0, c * TJ : (c + 1) * TJ].reshape(NB, 128).T
        parcel[:, NB] = sigma
        parcel[:, NB + 1] = weight[0]
        in_maps.append({"parcel": parcel, "mtoep": mtoep})

    res = run_bass_kernel_spmd(nc, in_maps, core_ids=list(range(N_CORES)))
    out = np.empty((1, T), dtype=np.float32)
    for c in range(N_CORES):
        out[0, c * TJ : (c + 1) * TJ] = res.results[c]["out"].T.reshape(-1)
    return out
